# revision 7
# baseline (speedup 1.0000x reference)
"""GCN (GCNConv + 3-layer MLP + log_softmax) on 8 Trainium2 NeuronCores.

Strategy (pull-mode message passing, fp16 data path):
  - Nodes are sharded 8 ways by destination; each core owns 12500 dst nodes
    (padded to 12544 = 98 tiles of 128 = 49 superblocks of 256).
  - Every core computes the full transformed feature table h = x @ W_gcn
    ([100352, 64] fp16, rows padded/permuted) into its own DRAM.
  - Edges (incl. self-loops) are partitioned by dst shard on the host,
    sorted by (dst superblock, src group, src), padded to 128-edge chunks
    at (superblock, group) granularity.  Padding slots carry idx=-1
    (skipped by the gather ucode) and norm=0.
  - Per 128-edge chunk the core gathers h[src] row-pairs with dma_gather
    (256 B = 2 fp16 rows; even/odd parity groups give a legal 256 B
    elem_step), builds a scaled one-hot S[e, j] = norm[e] *
    (dloc_sb[e] == j) over the whole 256-dst superblock with one fused
    tensor_scalar, and accumulates agg[64, 256] += msgs.T @ S on the
    tensor engine (PSUM), fp32 accumulation.
  - The MLP runs in feature-major (transposed) layout so all biases are
    per-partition activation biases; the last matmul flips back to
    node-major and log_softmax finishes on [128, 4] tiles.

The wall-clock is bounded by GPSIMD descriptor generation for the edge
gathers (~8.4 ns/row); everything else is sized to hide under it.
"""

import os
import sys

import numpy as np

sys.path.insert(0, "/opt/trn_rl_repo")

N = 100000
F = 256
H = 64
NCLS = 4
NCORES = 8
SHARD = 12500
SPAD = 12544          # 98 * 128
NT = SPAD // 128      # 98 dst tiles per core
NPAD = SPAD * NCORES  # 100352
NG = 4
GSZ = NPAD // NG      # 25088 rows per src group (< 2**15 for int16 idx)
TSB1 = 8              # phase-1 tiles per superblock; 1024-row blocks align
                      # with the half-table boundary (50176 = 49*1024)
TSB3 = 2              # phase-3 dst tiles per superblock (S width 256)
NSB = NT // TSB3      # 49 superblocks


def _host_prep(edge_index):
    """Partition/sort/pad edges; returns per-core device arrays + meta."""
    src = np.asarray(edge_index[0]).astype(np.int64)
    dst = np.asarray(edge_index[1]).astype(np.int64)
    deg = np.bincount(dst, minlength=N).astype(np.float64) + 1.0
    dinv = 1.0 / np.sqrt(deg)

    loop = np.arange(N, dtype=np.int64)
    srcA = np.concatenate([src, loop])
    dstA = np.concatenate([dst, loop])
    norm = (dinv[srcA] * dinv[dstA]).astype(np.float32)

    core = dstA // SHARD
    dl = dstA - core * SHARD
    sb = dl >> 8                                  # superblock (256 dsts)
    dloc = (dl & 255).astype(np.float32)          # position within sb
    srcp = (srcA // SHARD) * SPAD + (srcA % SHARD)   # padded global src id
    # h_all rows are stored partition-major per phase-1 superblock (so the
    # h write DMA is contiguous): node srcp lives at h_all row perm(srcp).
    blk = TSB1 * 128
    b = srcp // blk
    r = srcp - b * blk
    srcp = b * blk + (r % 128) * TSB1 + r // 128
    # groups: (half-table, row parity) — parity gives the 256 B-aligned
    # elem_step view (rows at stride 2) needed for fp16 pair gathers
    half = srcp // (2 * GSZ)
    w = srcp - half * (2 * GSZ)
    grp = half * 2 + (w & 1)
    idx16 = (w >> 1).astype(np.int16)

    key = ((core * NSB + sb) * NG + grp)
    order = np.argsort(key * np.int64(NPAD) + srcp, kind="stable")
    key_s = key[order]
    idx_s = idx16[order]
    dloc_s = dloc[order]
    norm_s = norm[order]

    cnt = np.bincount(key, minlength=NCORES * NSB * NG).reshape(NCORES, NSB, NG)
    C = ((cnt.max(axis=0) + 127) // 128).astype(np.int64)      # [NSB, NG]
    starts = np.zeros(NCORES * NSB * NG + 1, dtype=np.int64)
    np.cumsum(cnt.reshape(-1), out=starts[1:])

    # stream layout: for sb: for g: C[sb,g] chunks of 128 edges
    col_of = np.zeros((NSB, NG), dtype=np.int64)
    sb_meta = []
    col = 0
    for s in range(NSB):
        colbase = col
        Ls = []
        goffs = []
        for g in range(NG):
            goffs.append(col - colbase)
            col_of[s, g] = col
            col += C[s, g]
            Ls.append(int(128 * C[s, g]))
        sb_meta.append(dict(sb=s, colbase=int(colbase),
                            totc=int(col - colbase), L=Ls, goff=goffs))
    TOTC = int(col)
    TOT = TOTC * 128

    idx_streams, dloc_streams, norm_streams = [], [], []
    for c in range(NCORES):
        si = np.zeros(TOT, dtype=np.int16)        # pad: re-gather row 0
        sd = np.full(TOT, -1.0, dtype=np.float32)
        sn = np.zeros(TOT, dtype=np.float32)
        for s in range(NSB):
            for g in range(NG):
                k = (c * NSB + s) * NG + g
                n = cnt[c, s, g]
                if n == 0:
                    continue
                a = starts[k]
                o = col_of[s, g] * 128
                si[o:o + n] = idx_s[a:a + n]
                sd[o:o + n] = dloc_s[a:a + n]
                sn[o:o + n] = norm_s[a:a + n]
        idx_streams.append(np.tile(np.ascontiguousarray(
            si.reshape(-1, 16).T), (8, 1)))                       # [128, TOT/16]
        dloc_streams.append(np.ascontiguousarray(sd.reshape(-1, 128).T))
        norm_streams.append(np.ascontiguousarray(sn.reshape(-1, 128).T))
    meta = dict(C=C, sb_meta=sb_meta, TOTC=TOTC, TOT=TOT)
    return idx_streams, dloc_streams, norm_streams, meta


def _build_nc(meta):
    import concourse.bacc as bacc
    import concourse.mybir as mybir
    import concourse.tile as tile
    from concourse import library_config

    f32 = mybir.dt.float32
    f16 = mybir.dt.float16
    i16 = mybir.dt.int16
    AF = mybir.ActivationFunctionType
    ALU = mybir.AluOpType
    TOTC, TOT = meta["TOTC"], meta["TOT"]
    C, sb_meta = meta["C"], meta["sb_meta"]
    W3S = TSB3 * 128      # 256, S width

    nc = bacc.Bacc("TRN2")
    xT = nc.dram_tensor("xT", [F, NPAD], f16, kind="ExternalInput")
    wg = nc.dram_tensor("wg", [F, H], f16, kind="ExternalInput")
    w1 = nc.dram_tensor("w1", [64, 32], f32, kind="ExternalInput")
    w2 = nc.dram_tensor("w2", [32, 16], f32, kind="ExternalInput")
    w3 = nc.dram_tensor("w3", [16, 4], f32, kind="ExternalInput")
    bg = nc.dram_tensor("bg", [64, 1], f32, kind="ExternalInput")
    b1 = nc.dram_tensor("b1", [32, 1], f32, kind="ExternalInput")
    b2 = nc.dram_tensor("b2", [16, 1], f32, kind="ExternalInput")
    b3r = nc.dram_tensor("b3r", [1, 4], f32, kind="ExternalInput")
    iotam = nc.dram_tensor("iotam", [128, W3S], f16, kind="ExternalInput")
    onesr = nc.dram_tensor("onesr", [1, 128], f32, kind="ExternalInput")
    idxT = nc.dram_tensor("idx", [128, TOT // 16], i16, kind="ExternalInput")
    dlocT = nc.dram_tensor("dloc", [128, TOTC], f32, kind="ExternalInput")
    nrmT = nc.dram_tensor("nrm", [128, TOTC], f32, kind="ExternalInput")
    outT = nc.dram_tensor("out", [SPAD, NCLS], f32, kind="ExternalOutput")

    NT1 = NPAD // 128  # 784 phase-1 tiles
    sb1 = [list(range(s, min(s + TSB1, NT1))) for s in range(0, NT1, TSB1)]
    # per-pass (groups 0-1 / groups 2-3) chunk-count maxima for tile sizing
    maxc0 = max(m["goff"][2] for m in sb_meta)
    maxc1 = max(m["totc"] - m["goff"][2] for m in sb_meta)
    maxc = max(maxc0, maxc1)

    with tile.TileContext(nc) as tc:
        with tc.tile_pool(name="const", bufs=1) as cp, \
             tc.tile_pool(name="dram", bufs=1, space="DRAM") as dram:
            # +2 guard rows: the last odd-parity pair descriptor reads one
            # row past the half-table
            h01 = dram.tile([2 * GSZ + 2, H], f16, tag="h01")
            h23 = dram.tile([2 * GSZ + 2, H], f16, tag="h23")
            nc.gpsimd.load_library(library_config.mlp)

            wg0 = cp.tile([128, H], f16, tag="wg0")
            wg1 = cp.tile([128, H], f16, tag="wg1")
            nc.sync.dma_start(wg0[:], wg[0:128, :])
            nc.sync.dma_start(wg1[:], wg[128:256, :])
            w1s = cp.tile([64, 32], f32, tag="w1s")
            w2s = cp.tile([32, 16], f32, tag="w2s")
            w3s = cp.tile([16, 4], f32, tag="w3s")
            bgs = cp.tile([64, 1], f32, tag="bgs")
            b1s = cp.tile([32, 1], f32, tag="b1s")
            b2s = cp.tile([16, 1], f32, tag="b2s")
            b3s = cp.tile([1, 4], f32, tag="b3s")
            iots = cp.tile([128, W3S], f16, tag="iots")
            ones = cp.tile([1, 128], f32, tag="ones")
            for t_, d_ in ((w1s, w1), (w2s, w2), (w3s, w3), (bgs, bg),
                           (b1s, b1), (b2s, b2), (b3s, b3r), (iots, iotam),
                           (ones, onesr)):
                nc.sync.dma_start(t_[:], d_[:, :])

            with tc.tile_pool(name="p1", bufs=2) as p1p, \
                 tc.tile_pool(name="ps1", bufs=2, space="PSUM") as ps1, \
                 tc.tile_pool(name="p3", bufs=3) as p3p, \
                 tc.tile_pool(name="gb", bufs=3) as gbp, \
                 tc.tile_pool(name="sp", bufs=6) as sp, \
                 tc.tile_pool(name="ep", bufs=3) as ep, \
                 tc.tile_pool(name="oa", bufs=1) as oap, \
                 tc.tile_pool(name="agg", bufs=2, space="PSUM") as aggp, \
                 tc.tile_pool(name="mlp", bufs=3, space="PSUM") as mlpp:
                # gbuf ring buffers hold stale data where the gather skips
                # padding rows (idx=-1); scrub once so no NaN enters PSUM
                # through a zero S column (0*NaN = NaN).
                for _ in range(3):
                    gz = gbp.tile([128, maxc, 128], f16, tag="gbuf")
                    nc.vector.memset(gz[:], 0.0)

                # -------- phase 1: h = x @ W_gcn, halves written in order ---
                nhalf = len(sb1) // 2
                for bi, tiles in enumerate(sb1):
                    T = len(tiles)
                    t0 = tiles[0]
                    xt0 = p1p.tile([128, TSB1 * 128], f16, tag="xt0")
                    xt1 = p1p.tile([128, TSB1 * 128], f16, tag="xt1")
                    nc.sync.dma_start(
                        xt0[:, :T * 128], xT[0:128, t0 * 128:(t0 + T) * 128])
                    nc.sync.dma_start(
                        xt1[:, :T * 128], xT[128:256, t0 * 128:(t0 + T) * 128])
                    hsb = p1p.tile([128, TSB1, H], f16, tag="hsb")
                    for i in range(T):
                        ps = ps1.tile([128, H], f32, tag="hps")
                        nc.tensor.matmul(ps[:], xt0[:, i * 128:(i + 1) * 128],
                                         wg0[:], start=True, stop=False)
                        nc.tensor.matmul(ps[:], xt1[:, i * 128:(i + 1) * 128],
                                         wg1[:], start=False, stop=True)
                        nc.vector.tensor_copy(hsb[:, i, :], ps[:])
                    hP = h01 if bi < nhalf else h23
                    r0 = (bi if bi < nhalf else bi - nhalf) * TSB1 * 128
                    # partition-major row order -> contiguous 1 KB runs
                    nc.sync.dma_start(
                        hP[r0:r0 + T * 128, :]
                        .rearrange("(p t) f -> p t f", p=128),
                        hsb[:, :T, :])

                # -------- phase 3: two passes (groups 0-1, then 2-3) --------
                outacc = oap.tile([128, NT, NCLS], f32, tag="outacc")
                accT = oap.tile([64, NT * 128], f32, tag="accT")
                for pas in (0, 1):
                    hP = h01 if pas == 0 else h23
                    gl = 2 * pas
                    for m in sb_meta:
                        s = m["sb"]
                        pco = m["goff"][gl]                  # pass col offset
                        pend = m["totc"] if pas else m["goff"][2]
                        ptc = pend - pco                     # pass chunk count
                        cb = m["colbase"] + pco              # global col base
                        idxsb = p3p.tile([128, maxc * 8], i16, tag="idx")
                        nc.sync.dma_start(idxsb[:, :ptc * 8],
                                          idxT[:, cb * 8:(cb + ptc) * 8])
                        dlsb = p3p.tile([128, maxc], f32, tag="dl")
                        nrsb = p3p.tile([128, maxc], f32, tag="nr")
                        nc.sync.dma_start(dlsb[:, :ptc],
                                          dlocT[:, cb:cb + ptc])
                        nc.sync.dma_start(nrsb[:, :ptc],
                                          nrmT[:, cb:cb + ptc])
                        gbuf = gbp.tile([128, maxc, 128], f16, tag="gbuf")
                        for g in (gl, gl + 1):
                            L = m["L"][g]
                            go = m["goff"][g] - pco
                            # pair view: row k = table rows [2k+par, 2k+par+1]
                            # (256 B descriptor, inner dim = elem_size = 128)
                            par = g & 1
                            hV = hP[:].rearrange("r f -> (r f)")[
                                par * H:par * H + GSZ * 2 * H].rearrange(
                                "(r ff) -> r ff", ff=2 * H)
                            # SWDGE ring caps one gather at ~1024 idxs
                            for k in range(0, L, 1024):
                                ni = min(1024, L - k)
                                c0 = go + k // 128
                                nc.gpsimd.dma_gather(
                                    gbuf[:, c0:c0 + ni // 128, :],
                                    hV,
                                    idxsb[:, c0 * 8:(c0 + ni // 128) * 8],
                                    ni, ni, 2 * H, elem_step=2 * H)
                        agg = aggp.tile([64, W3S], f32, tag="agg")
                        if ptc == 0:
                            nc.vector.memset(agg[:], 0.0)
                        for j in range(ptc):
                            S = sp.tile([128, W3S], f16, tag="S")
                            nc.vector.tensor_scalar(
                                S[:], iots[:], dlsb[:, j:j + 1],
                                nrsb[:, j:j + 1],
                                op0=ALU.is_equal, op1=ALU.mult)
                            nc.tensor.matmul(
                                agg[:], gbuf[:, j, 0:H], S[:],
                                start=(j == 0),
                                stop=(j == ptc - 1))
                        if pas == 0:
                            nc.vector.tensor_copy(
                                accT[:, s * W3S:(s + 1) * W3S], agg[:])
                            continue
                        t0p = ep.tile([64, W3S], f32, tag="t0p")
                        nc.vector.tensor_add(
                            t0p[:], accT[:, s * W3S:(s + 1) * W3S], agg[:])
                        for ti in range(TSB3):
                            t = s * TSB3 + ti
                            t0s = ep.tile([64, 128], f32, tag="t0")
                            nc.scalar.activation(
                                t0s[:], t0p[:, ti * 128:(ti + 1) * 128],
                                AF.Relu, bias=bgs[:])
                            pm1 = mlpp.tile([32, 128], f32, tag="pm")
                            nc.tensor.matmul(pm1[:], w1s[:], t0s[:],
                                             start=True, stop=True)
                            t1s = ep.tile([32, 128], f32, tag="t1")
                            nc.scalar.activation(t1s[:], pm1[:], AF.Relu,
                                                 bias=b1s[:])
                            pm2 = mlpp.tile([16, 128], f32, tag="pm")
                            nc.tensor.matmul(pm2[:], w2s[:], t1s[:],
                                             start=True, stop=True)
                            t2s = ep.tile([16, 128], f32, tag="t2")
                            nc.scalar.activation(t2s[:], pm2[:], AF.Relu,
                                                 bias=b2s[:])
                            pm3 = mlpp.tile([128, NCLS], f32, tag="pm")
                            nc.tensor.matmul(pm3[:], t2s[:], w3s[:],
                                             start=True, stop=False)
                            nc.tensor.matmul(pm3[:], ones[:], b3s[:],
                                             start=False, stop=True)
                            nmax = ep.tile([128, 1], f32, tag="nmax")
                            nc.vector.tensor_reduce(nmax[:], pm3[:],
                                                    axis=mybir.AxisListType.X,
                                                    op=ALU.max, negate=True)
                            esb = ep.tile([128, NCLS], f32, tag="esb")
                            ssum = ep.tile([128, 1], f32, tag="ssum")
                            nc.scalar.activation(esb[:], pm3[:], AF.Exp,
                                                 bias=nmax[:],
                                                 accum_out=ssum[:])
                            lsb = ep.tile([128, 1], f32, tag="lsb")
                            nc.scalar.activation(lsb[:], ssum[:], AF.Ln)
                            nc.vector.tensor_scalar(
                                outacc[:, t, :], pm3[:], nmax[:], lsb[:],
                                op0=ALU.add, op1=ALU.subtract)
                nc.sync.dma_start(
                    outT[:, :].rearrange("(t p) c -> p t c", p=128),
                    outacc[:])
    nc.compile()
    return nc


def kernel(x, edge_index, W_gcn, b_gcn, W1, b1, W2, b2, W3, b3,
           _trace=False, _tmpdir=None):
    from concourse.bass_utils import run_bass_kernel_spmd

    x = np.asarray(x, dtype=np.float32)
    idx_streams, dloc_streams, norm_streams, meta = _host_prep(edge_index)
    nc = _build_nc(meta)

    xTp = np.zeros((F, NPAD), dtype=np.float16)
    xt = np.ascontiguousarray(x.T).astype(np.float16)
    for c in range(NCORES):
        xTp[:, c * SPAD:c * SPAD + SHARD] = xt[:, c * SHARD:(c + 1) * SHARD]
    iot = np.tile(np.arange(TSB3 * 128, dtype=np.float16), (128, 1))
    common = {
        "xT": xTp,
        "wg": np.asarray(W_gcn, np.float16),
        "w1": np.asarray(W1, np.float32),
        "w2": np.asarray(W2, np.float32),
        "w3": np.asarray(W3, np.float32),
        "bg": np.asarray(b_gcn, np.float32).reshape(64, 1),
        "b1": np.asarray(b1, np.float32).reshape(32, 1),
        "b2": np.asarray(b2, np.float32).reshape(16, 1),
        "b3r": np.asarray(b3, np.float32).reshape(1, 4),
        "iotam": iot,
        "onesr": np.ones((1, 128), dtype=np.float32),
    }
    in_maps = []
    for c in range(NCORES):
        m = dict(common)
        m["idx"] = idx_streams[c]
        m["dloc"] = dloc_streams[c]
        m["nrm"] = norm_streams[c]
        in_maps.append(m)

    res = run_bass_kernel_spmd(nc, in_maps, core_ids=list(range(NCORES)),
                               trace=_trace, tmpdir=_tmpdir)
    out = np.concatenate(
        [res.results[c]["out"][:SHARD] for c in range(NCORES)], axis=0)
    if _trace:
        kernel.last_exec_time_ns = res.exec_time_ns
    return out


kernel.last_exec_time_ns = None


# revision 10
# speedup vs baseline: 1.1989x; 1.1989x over previous
"""GCN (GCNConv + 3-layer MLP + log_softmax) on 8 Trainium2 NeuronCores.

Strategy (pull-mode message passing, fp16 data path):
  - Nodes are sharded 8 ways by destination; each core owns 12500 dst nodes
    (padded to 12544 = 98 tiles of 128 = 49 superblocks of 256).
  - Every core computes the full transformed feature table h = x @ W_gcn
    ([100352, 64] fp16, rows padded/permuted) into its own DRAM.
  - Edges (incl. self-loops) are partitioned by dst shard on the host,
    sorted by (dst superblock, src group, src), padded to 128-edge chunks
    at (superblock, group) granularity.  Padding slots carry idx=-1
    (skipped by the gather ucode) and norm=0.
  - Per 128-edge chunk the core gathers h[src] row-pairs with dma_gather
    (256 B = 2 fp16 rows; even/odd parity groups give a legal 256 B
    elem_step), builds a scaled one-hot S[e, j] = norm[e] *
    (dloc_sb[e] == j) over the whole 256-dst superblock with one fused
    tensor_scalar, and accumulates agg[64, 256] += msgs.T @ S on the
    tensor engine (PSUM), fp32 accumulation.
  - The MLP runs in feature-major (transposed) layout so all biases are
    per-partition activation biases; the last matmul flips back to
    node-major and log_softmax finishes on [128, 4] tiles.

The wall-clock is bounded by GPSIMD descriptor generation for the edge
gathers (~8.4 ns/row); everything else is sized to hide under it.
"""

import os
import sys

import numpy as np

sys.path.insert(0, "/opt/trn_rl_repo")

N = 100000
F = 256
H = 64
NCLS = 4
NCORES = 8
SHARD = 12500
SPAD = 12544          # 98 * 128
NT = SPAD // 128      # 98 dst tiles per core
NPAD = SPAD * NCORES  # 100352
NG = 4
GSZ = NPAD // NG      # 25088 rows per src group (< 2**15 for int16 idx)
TSB1 = 8              # phase-1 tiles per superblock; 1024-row blocks align
                      # with the half-table boundary (50176 = 49*1024)
TSB3 = 4              # phase-3 dst tiles per superblock (S width 512)
NSB = (NT + TSB3 - 1) // TSB3   # 25 superblocks (last has 2 tiles)


def _host_prep(edge_index):
    """Partition/sort/pad edges; returns per-core device arrays + meta."""
    src = np.asarray(edge_index[0]).astype(np.int64)
    dst = np.asarray(edge_index[1]).astype(np.int64)
    deg = np.bincount(dst, minlength=N).astype(np.float64) + 1.0
    dinv = 1.0 / np.sqrt(deg)

    loop = np.arange(N, dtype=np.int64)
    srcA = np.concatenate([src, loop])
    dstA = np.concatenate([dst, loop])
    core = dstA // SHARD
    dl = dstA - core * SHARD
    sb = dl // (TSB3 * 128)                       # superblock (512 dsts)
    dloc = (dl - sb * TSB3 * 128).astype(np.float32)  # position within sb
    srcp = (srcA // SHARD) * SPAD + (srcA % SHARD)   # padded global src id
    # h_all rows are stored partition-major per phase-1 superblock (so the
    # h write DMA is contiguous): node srcp lives at h_all row perm(srcp).
    blk = TSB1 * 128
    b = srcp // blk
    r = srcp - b * blk
    srcp = b * blk + (r % 128) * TSB1 + r // 128
    # groups: (half-table, row parity) — parity gives the 256 B-aligned
    # elem_step view (rows at stride 2) needed for fp16 pair gathers
    half = srcp // (2 * GSZ)
    w = srcp - half * (2 * GSZ)
    grp = half * 2 + (w & 1)
    idx16 = (w >> 1).astype(np.int16)

    key = ((core * NSB + sb) * NG + grp)
    order = np.argsort(key * np.int64(NPAD) + srcp, kind="stable")
    idx_s = idx16[order]
    dloc_s = dloc[order]

    cnt = np.bincount(key, minlength=NCORES * NSB * NG).reshape(NCORES, NSB, NG)
    C = ((cnt.max(axis=0) + 127) // 128).astype(np.int64)      # [NSB, NG]
    starts = np.zeros(NCORES * NSB * NG + 1, dtype=np.int64)
    np.cumsum(cnt.reshape(-1), out=starts[1:])

    # stream layout: for sb: for g: C[sb,g] chunks of 128 edges
    col_of = np.zeros((NSB, NG), dtype=np.int64)
    sb_meta = []
    col = 0
    for s in range(NSB):
        colbase = col
        Ls = []
        goffs = []
        for g in range(NG):
            goffs.append(col - colbase)
            col_of[s, g] = col
            col += C[s, g]
            Ls.append(int(128 * C[s, g]))
        sb_meta.append(dict(sb=s, colbase=int(colbase),
                            totc=int(col - colbase), L=Ls, goff=goffs))
    TOTC = int(col)
    TOT = TOTC * 128

    idx_streams, dloc_streams = [], []
    for c in range(NCORES):
        si = np.zeros(TOT, dtype=np.int16)        # pad: re-gather row 0
        sd = np.full(TOT, -1.0, dtype=np.float32)
        for s in range(NSB):
            for g in range(NG):
                k = (c * NSB + s) * NG + g
                n = cnt[c, s, g]
                if n == 0:
                    continue
                a = starts[k]
                o = col_of[s, g] * 128
                si[o:o + n] = idx_s[a:a + n]
                sd[o:o + n] = dloc_s[a:a + n]
        idx_streams.append(np.tile(np.ascontiguousarray(
            si.reshape(-1, 16).T), (8, 1)))                       # [128, TOT/16]
        dloc_streams.append(np.ascontiguousarray(sd.reshape(-1, 128).T))
    # dinv by phase-1 table layout: dinvP[p, b*8+t] = dinv(node b*1024+t*128+p)
    dinv_pad = np.zeros(NPAD, dtype=np.float32)
    for c in range(NCORES):
        dinv_pad[c * SPAD:c * SPAD + SHARD] = dinv[c * SHARD:(c + 1) * SHARD]
    NT1 = NPAD // 128
    dP = dinv_pad.reshape(NT1 // TSB1, TSB1, 128)     # [b, t, p]
    dinvP = np.ascontiguousarray(
        dP.transpose(2, 0, 1).reshape(128, NT1)).astype(np.float32)
    # per-core dst dinv, replicated over 64 feature partitions
    dinvD = []
    for c in range(NCORES):
        dd = np.zeros(SPAD, dtype=np.float32)
        dd[:SHARD] = dinv[c * SHARD:(c + 1) * SHARD]
        dinvD.append(np.ascontiguousarray(np.tile(dd, (64, 1))))
    meta = dict(C=C, sb_meta=sb_meta, TOTC=TOTC, TOT=TOT)
    return idx_streams, dloc_streams, dinvP, dinvD, meta


def _build_nc(meta):
    import concourse.bacc as bacc
    import concourse.mybir as mybir
    import concourse.tile as tile
    from concourse import library_config

    f32 = mybir.dt.float32
    f16 = mybir.dt.float16
    i16 = mybir.dt.int16
    AF = mybir.ActivationFunctionType
    ALU = mybir.AluOpType
    TOTC, TOT = meta["TOTC"], meta["TOT"]
    C, sb_meta = meta["C"], meta["sb_meta"]
    W3S = TSB3 * 128      # 256, S width

    nc = bacc.Bacc("TRN2")
    xT = nc.dram_tensor("xT", [F, NPAD], f16, kind="ExternalInput")
    wg = nc.dram_tensor("wg", [F, H], f16, kind="ExternalInput")
    w1 = nc.dram_tensor("w1", [64, 32], f32, kind="ExternalInput")
    w2 = nc.dram_tensor("w2", [32, 16], f32, kind="ExternalInput")
    w3 = nc.dram_tensor("w3", [16, 4], f32, kind="ExternalInput")
    bg = nc.dram_tensor("bg", [64, 1], f32, kind="ExternalInput")
    b1 = nc.dram_tensor("b1", [32, 1], f32, kind="ExternalInput")
    b2 = nc.dram_tensor("b2", [16, 1], f32, kind="ExternalInput")
    b3r = nc.dram_tensor("b3r", [1, 4], f32, kind="ExternalInput")
    iotam = nc.dram_tensor("iotam", [128, W3S], f16, kind="ExternalInput")
    onesr = nc.dram_tensor("onesr", [1, 128], f32, kind="ExternalInput")
    idxT = nc.dram_tensor("idx", [128, TOT // 16], i16, kind="ExternalInput")
    dlocT = nc.dram_tensor("dloc", [128, TOTC], f32, kind="ExternalInput")
    dinvPT = nc.dram_tensor("dinvP", [128, NPAD // 128], f32,
                            kind="ExternalInput")
    dinvDT = nc.dram_tensor("dinvD", [64, SPAD], f32, kind="ExternalInput")
    outT = nc.dram_tensor("out", [SPAD, NCLS], f32, kind="ExternalOutput")

    NT1 = NPAD // 128  # 784 phase-1 tiles
    sb1 = [list(range(s, min(s + TSB1, NT1))) for s in range(0, NT1, TSB1)]
    # per-pass (groups 0-1 / groups 2-3) chunk-count maxima for tile sizing
    maxc0 = max(m["goff"][2] for m in sb_meta)
    maxc1 = max(m["totc"] - m["goff"][2] for m in sb_meta)
    maxc = max(maxc0, maxc1)

    with tile.TileContext(nc) as tc:
        with tc.tile_pool(name="const", bufs=1) as cp, \
             tc.tile_pool(name="dram", bufs=1, space="DRAM") as dram:
            # +2 guard rows: the last odd-parity pair descriptor reads one
            # row past the half-table
            h01 = dram.tile([2 * GSZ + 2, H], f16, tag="h01")
            h23 = dram.tile([2 * GSZ + 2, H], f16, tag="h23")
            nc.gpsimd.load_library(library_config.mlp)

            wg0 = cp.tile([128, H], f16, tag="wg0")
            wg1 = cp.tile([128, H], f16, tag="wg1")
            nc.sync.dma_start(wg0[:], wg[0:128, :])
            nc.sync.dma_start(wg1[:], wg[128:256, :])
            w1s = cp.tile([64, 32], f32, tag="w1s")
            w2s = cp.tile([32, 16], f32, tag="w2s")
            w3s = cp.tile([16, 4], f32, tag="w3s")
            bgs = cp.tile([64, 1], f32, tag="bgs")
            b1s = cp.tile([32, 1], f32, tag="b1s")
            b2s = cp.tile([16, 1], f32, tag="b2s")
            b3s = cp.tile([1, 4], f32, tag="b3s")
            iots = cp.tile([128, W3S], f16, tag="iots")
            dps = cp.tile([128, NPAD // 128], f32, tag="dps")
            nc.sync.dma_start(dps[:], dinvPT[:, :])
            ones = cp.tile([1, 128], f32, tag="ones")
            for t_, d_ in ((w1s, w1), (w2s, w2), (w3s, w3), (bgs, bg),
                           (b1s, b1), (b2s, b2), (b3s, b3r), (iots, iotam),
                           (ones, onesr)):
                nc.sync.dma_start(t_[:], d_[:, :])

            with tc.tile_pool(name="p1", bufs=2) as p1p, \
                 tc.tile_pool(name="ps1", bufs=2, space="PSUM") as ps1, \
                 tc.tile_pool(name="p3", bufs=3) as p3p, \
                 tc.tile_pool(name="gb", bufs=3) as gbp, \
                 tc.tile_pool(name="sp", bufs=6) as sp, \
                 tc.tile_pool(name="ep", bufs=3) as ep, \
                 tc.tile_pool(name="oa", bufs=1) as oap, \
                 tc.tile_pool(name="agg", bufs=2, space="PSUM") as aggp, \
                 tc.tile_pool(name="mlp", bufs=3, space="PSUM") as mlpp:
                # -------- phase 1: h = x @ W_gcn, halves written in order ---
                nhalf = len(sb1) // 2
                for bi, tiles in enumerate(sb1):
                    T = len(tiles)
                    t0 = tiles[0]
                    xt0 = p1p.tile([128, TSB1 * 128], f16, tag="xt0")
                    xt1 = p1p.tile([128, TSB1 * 128], f16, tag="xt1")
                    nc.sync.dma_start(
                        xt0[:, :T * 128], xT[0:128, t0 * 128:(t0 + T) * 128])
                    nc.sync.dma_start(
                        xt1[:, :T * 128], xT[128:256, t0 * 128:(t0 + T) * 128])
                    hsb = p1p.tile([128, TSB1, H], f16, tag="hsb")
                    for i in range(T):
                        ps = ps1.tile([128, H], f32, tag="hps")
                        nc.tensor.matmul(ps[:], xt0[:, i * 128:(i + 1) * 128],
                                         wg0[:], start=True, stop=False)
                        nc.tensor.matmul(ps[:], xt1[:, i * 128:(i + 1) * 128],
                                         wg1[:], start=False, stop=True)
                        nc.scalar.activation(hsb[:, i, :], ps[:], AF.Copy,
                                             scale=dps[:, t0 + i:t0 + i + 1])
                    hP = h01 if bi < nhalf else h23
                    r0 = (bi if bi < nhalf else bi - nhalf) * TSB1 * 128
                    # partition-major row order -> contiguous 1 KB runs
                    nc.sync.dma_start(
                        hP[r0:r0 + T * 128, :]
                        .rearrange("(p t) f -> p t f", p=128),
                        hsb[:, :T, :])

                # -------- phase 3: two passes (groups 0-1, then 2-3) --------
                outacc = oap.tile([128, NT, NCLS], f32, tag="outacc")
                accT = oap.tile([64, NT * 128], f32, tag="accT")
                for pas in (0, 1):
                    hP = h01 if pas == 0 else h23
                    gl = 2 * pas
                    for m in sb_meta:
                        s = m["sb"]
                        wsb = min(TSB3, NT - s * TSB3) * 128
                        pco = m["goff"][gl]                  # pass col offset
                        pend = m["totc"] if pas else m["goff"][2]
                        ptc = pend - pco                     # pass chunk count
                        cb = m["colbase"] + pco              # global col base
                        idxsb = p3p.tile([128, maxc * 8], i16, tag="idx")
                        nc.sync.dma_start(idxsb[:, :ptc * 8],
                                          idxT[:, cb * 8:(cb + ptc) * 8])
                        dlsb = p3p.tile([128, maxc], f32, tag="dl")
                        nc.sync.dma_start(dlsb[:, :ptc],
                                          dlocT[:, cb:cb + ptc])
                        gbuf = gbp.tile([128, maxc, 128], f16, tag="gbuf")
                        for g in (gl, gl + 1):
                            L = m["L"][g]
                            go = m["goff"][g] - pco
                            # pair view: row k = table rows [2k+par, 2k+par+1]
                            # (256 B descriptor, inner dim = elem_size = 128)
                            par = g & 1
                            hV = hP[:].rearrange("r f -> (r f)")[
                                par * H:par * H + GSZ * 2 * H].rearrange(
                                "(r ff) -> r ff", ff=2 * H)
                            # SWDGE ring caps one gather at ~1024 idxs
                            for k in range(0, L, 1024):
                                ni = min(1024, L - k)
                                c0 = go + k // 128
                                nc.gpsimd.dma_gather(
                                    gbuf[:, c0:c0 + ni // 128, :],
                                    hV,
                                    idxsb[:, c0 * 8:(c0 + ni // 128) * 8],
                                    ni, ni, 2 * H, elem_step=2 * H)
                        agg = aggp.tile([64, W3S], f32, tag="agg")
                        if ptc == 0:
                            nc.vector.memset(agg[:], 0.0)
                        for j in range(ptc):
                            S = sp.tile([128, W3S], f16, tag="S")
                            nc.vector.tensor_scalar(
                                S[:], iots[:], dlsb[:, j:j + 1], None,
                                op0=ALU.is_equal)
                            nc.tensor.matmul(
                                agg[:], gbuf[:, j, 0:H], S[:],
                                start=(j == 0),
                                stop=(j == ptc - 1))
                        if pas == 0:
                            nc.vector.tensor_copy(
                                accT[:, s * W3S:s * W3S + wsb], agg[:, :wsb])
                            continue
                        ddsb = ep.tile([64, W3S], f32, tag="dd")
                        nc.sync.dma_start(
                            ddsb[:, :wsb], dinvDT[:, s * W3S:s * W3S + wsb])
                        t0p = ep.tile([64, W3S], f32, tag="t0p")
                        nc.vector.tensor_add(
                            t0p[:, :wsb], accT[:, s * W3S:s * W3S + wsb],
                            agg[:, :wsb])
                        t0m = ep.tile([64, W3S], f32, tag="t0m")
                        nc.vector.tensor_mul(t0m[:, :wsb], t0p[:, :wsb],
                                             ddsb[:, :wsb])
                        for ti in range(min(TSB3, NT - s * TSB3)):
                            t = s * TSB3 + ti
                            t0s = ep.tile([64, 128], f32, tag="t0")
                            nc.scalar.activation(
                                t0s[:], t0m[:, ti * 128:(ti + 1) * 128],
                                AF.Relu, bias=bgs[:])
                            pm1 = mlpp.tile([32, 128], f32, tag="pm")
                            nc.tensor.matmul(pm1[:], w1s[:], t0s[:],
                                             start=True, stop=True)
                            t1s = ep.tile([32, 128], f32, tag="t1")
                            nc.scalar.activation(t1s[:], pm1[:], AF.Relu,
                                                 bias=b1s[:])
                            pm2 = mlpp.tile([16, 128], f32, tag="pm")
                            nc.tensor.matmul(pm2[:], w2s[:], t1s[:],
                                             start=True, stop=True)
                            t2s = ep.tile([16, 128], f32, tag="t2")
                            nc.scalar.activation(t2s[:], pm2[:], AF.Relu,
                                                 bias=b2s[:])
                            pm3 = mlpp.tile([128, NCLS], f32, tag="pm")
                            nc.tensor.matmul(pm3[:], t2s[:], w3s[:],
                                             start=True, stop=False)
                            nc.tensor.matmul(pm3[:], ones[:], b3s[:],
                                             start=False, stop=True)
                            nmax = ep.tile([128, 1], f32, tag="nmax")
                            nc.vector.tensor_reduce(nmax[:], pm3[:],
                                                    axis=mybir.AxisListType.X,
                                                    op=ALU.max, negate=True)
                            esb = ep.tile([128, NCLS], f32, tag="esb")
                            ssum = ep.tile([128, 1], f32, tag="ssum")
                            nc.scalar.activation(esb[:], pm3[:], AF.Exp,
                                                 bias=nmax[:],
                                                 accum_out=ssum[:])
                            lsb = ep.tile([128, 1], f32, tag="lsb")
                            nc.scalar.activation(lsb[:], ssum[:], AF.Ln)
                            nc.vector.tensor_scalar(
                                outacc[:, t, :], pm3[:], nmax[:], lsb[:],
                                op0=ALU.add, op1=ALU.subtract)
                nc.sync.dma_start(
                    outT[:, :].rearrange("(t p) c -> p t c", p=128),
                    outacc[:])
    nc.compile()
    return nc


def kernel(x, edge_index, W_gcn, b_gcn, W1, b1, W2, b2, W3, b3,
           _trace=False, _tmpdir=None):
    from concourse.bass_utils import run_bass_kernel_spmd

    x = np.asarray(x, dtype=np.float32)
    idx_streams, dloc_streams, dinvP, dinvD, meta = _host_prep(edge_index)
    nc = _build_nc(meta)

    xTp = np.zeros((F, NPAD), dtype=np.float16)
    xt = np.ascontiguousarray(x.T).astype(np.float16)
    for c in range(NCORES):
        xTp[:, c * SPAD:c * SPAD + SHARD] = xt[:, c * SHARD:(c + 1) * SHARD]
    iot = np.tile(np.arange(TSB3 * 128, dtype=np.float16), (128, 1))
    common = {
        "xT": xTp,
        "wg": np.asarray(W_gcn, np.float16),
        "w1": np.asarray(W1, np.float32),
        "w2": np.asarray(W2, np.float32),
        "w3": np.asarray(W3, np.float32),
        "bg": np.asarray(b_gcn, np.float32).reshape(64, 1),
        "b1": np.asarray(b1, np.float32).reshape(32, 1),
        "b2": np.asarray(b2, np.float32).reshape(16, 1),
        "b3r": np.asarray(b3, np.float32).reshape(1, 4),
        "iotam": iot,
        "onesr": np.ones((1, 128), dtype=np.float32),
        "dinvP": dinvP,
    }
    in_maps = []
    for c in range(NCORES):
        m = dict(common)
        m["idx"] = idx_streams[c]
        m["dloc"] = dloc_streams[c]
        m["dinvD"] = dinvD[c]
        in_maps.append(m)

    res = run_bass_kernel_spmd(nc, in_maps, core_ids=list(range(NCORES)),
                               trace=_trace, tmpdir=_tmpdir)
    out = np.concatenate(
        [res.results[c]["out"][:SHARD] for c in range(NCORES)], axis=0)
    if _trace:
        kernel.last_exec_time_ns = res.exec_time_ns
    return out


kernel.last_exec_time_ns = None


# revision 11
# speedup vs baseline: 1.1991x; 1.0002x over previous
"""GCN (GCNConv + 3-layer MLP + log_softmax) on 8 Trainium2 NeuronCores.

Strategy (pull-mode message passing, fp16 data path):
  - Nodes are sharded 8 ways by destination; each core owns 12500 dst nodes
    (padded to 12544 = 98 tiles of 128 = 49 superblocks of 256).
  - Every core computes the full transformed feature table h = x @ W_gcn
    ([100352, 64] fp16, rows padded/permuted) into its own DRAM.
  - Edges (incl. self-loops) are partitioned by dst shard on the host,
    sorted by (dst superblock, src group, src), padded to 128-edge chunks
    at (superblock, group) granularity.  Padding slots carry idx=-1
    (skipped by the gather ucode) and norm=0.
  - Per 128-edge chunk the core gathers h[src] row-pairs with dma_gather
    (256 B = 2 fp16 rows; even/odd parity groups give a legal 256 B
    elem_step), builds a scaled one-hot S[e, j] = norm[e] *
    (dloc_sb[e] == j) over the whole 256-dst superblock with one fused
    tensor_scalar, and accumulates agg[64, 256] += msgs.T @ S on the
    tensor engine (PSUM), fp32 accumulation.
  - The MLP runs in feature-major (transposed) layout so all biases are
    per-partition activation biases; the last matmul flips back to
    node-major and log_softmax finishes on [128, 4] tiles.

The wall-clock is bounded by GPSIMD descriptor generation for the edge
gathers (~8.4 ns/row); everything else is sized to hide under it.
"""

import os
import sys

import numpy as np

sys.path.insert(0, "/opt/trn_rl_repo")

N = 100000
F = 256
H = 64
NCLS = 4
NCORES = 8
SHARD = 12500
SPAD = 12544          # 98 * 128
NT = SPAD // 128      # 98 dst tiles per core
NPAD = SPAD * NCORES  # 100352
NG = 4
GSZ = NPAD // NG      # 25088 rows per src group (< 2**15 for int16 idx)
TSB1 = 8              # phase-1 tiles per superblock; 1024-row blocks align
                      # with the half-table boundary (50176 = 49*1024)
TSB3 = 4              # phase-3 dst tiles per superblock (S width 512)
NSB = (NT + TSB3 - 1) // TSB3   # 25 superblocks (last has 2 tiles)


def _host_prep(edge_index):
    """Partition/sort/pad edges; returns per-core device arrays + meta."""
    src = np.asarray(edge_index[0]).astype(np.int64)
    dst = np.asarray(edge_index[1]).astype(np.int64)
    deg = np.bincount(dst, minlength=N).astype(np.float64) + 1.0
    dinv = 1.0 / np.sqrt(deg)

    loop = np.arange(N, dtype=np.int64)
    srcA = np.concatenate([src, loop])
    dstA = np.concatenate([dst, loop])
    is_self = np.zeros(srcA.shape[0], dtype=bool)
    is_self[src.shape[0]:] = True
    core = dstA // SHARD
    dl = dstA - core * SHARD
    sb = dl // (TSB3 * 128)                       # superblock (512 dsts)
    dloc = (dl - sb * TSB3 * 128).astype(np.float32)  # position within sb
    srcp = (srcA // SHARD) * SPAD + (srcA % SHARD)   # padded global src id
    # h_all rows are stored partition-major per phase-1 superblock (so the
    # h write DMA is contiguous): node srcp lives at h_all row perm(srcp).
    blk = TSB1 * 128
    b = srcp // blk
    r = srcp - b * blk
    srcp = b * blk + (r % 128) * TSB1 + r // 128
    # groups: (half-table, row parity) — parity gives the 256 B-aligned
    # elem_step view (rows at stride 2) needed for fp16 pair gathers
    half = srcp // (2 * GSZ)
    w = srcp - half * (2 * GSZ)
    grp = half * 2 + (w & 1)
    idx16 = (w >> 1).astype(np.int16)
    # self-loops: dedicated mod-4 stride-view cells (groups 4..7), gathered
    # from the full-table copy; perfectly balanced across cores
    grp = np.where(is_self, 4 + (srcp & 3), grp)
    idx16 = np.where(is_self, (srcp >> 2).astype(np.int16), idx16)

    NGT = 8
    key = ((core * NSB + sb) * NGT + grp)
    order = np.argsort(key * np.int64(NPAD) + srcp, kind="stable")
    idx_s = idx16[order]
    dloc_s = dloc[order]

    cnt = np.bincount(key, minlength=NCORES * NSB * NGT).reshape(
        NCORES, NSB, NGT)
    C = ((cnt.max(axis=0) + 127) // 128).astype(np.int64)      # [NSB, NG]
    starts = np.zeros(NCORES * NSB * NGT + 1, dtype=np.int64)
    np.cumsum(cnt.reshape(-1), out=starts[1:])

    # stream layout: for sb: for g: C[sb,g] chunks of 128 edges
    col_of = np.zeros((NSB, NGT), dtype=np.int64)
    sb_meta = []
    col = 0
    for s in range(NSB):
        colbase = col
        Ls = []
        goffs = []
        for g in range(NGT):
            goffs.append(col - colbase)
            col_of[s, g] = col
            col += C[s, g]
            Ls.append(int(128 * C[s, g]))
        sb_meta.append(dict(sb=s, colbase=int(colbase),
                            totc=int(col - colbase), L=Ls, goff=goffs))
    TOTC = int(col)
    TOT = TOTC * 128

    idx_streams, dloc_streams = [], []
    for c in range(NCORES):
        si = np.zeros(TOT, dtype=np.int16)        # pad: re-gather row 0
        sd = np.full(TOT, -1.0, dtype=np.float32)
        for s in range(NSB):
            for g in range(8):
                k = (c * NSB + s) * 8 + g
                n = cnt[c, s, g]
                if n == 0:
                    continue
                a = starts[k]
                o = col_of[s, g] * 128
                si[o:o + n] = idx_s[a:a + n]
                sd[o:o + n] = dloc_s[a:a + n]
        idx_streams.append(np.tile(np.ascontiguousarray(
            si.reshape(-1, 16).T), (8, 1)))                       # [128, TOT/16]
        dloc_streams.append(np.ascontiguousarray(sd.reshape(-1, 128).T))
    # dinv by phase-1 table layout: dinvP[p, b*8+t] = dinv(node b*1024+t*128+p)
    dinv_pad = np.zeros(NPAD, dtype=np.float32)
    for c in range(NCORES):
        dinv_pad[c * SPAD:c * SPAD + SHARD] = dinv[c * SHARD:(c + 1) * SHARD]
    NT1 = NPAD // 128
    dP = dinv_pad.reshape(NT1 // TSB1, TSB1, 128)     # [b, t, p]
    dinvP = np.ascontiguousarray(
        dP.transpose(2, 0, 1).reshape(128, NT1)).astype(np.float32)
    # per-core dst dinv, replicated over 64 feature partitions
    dinvD = []
    for c in range(NCORES):
        dd = np.zeros(SPAD, dtype=np.float32)
        dd[:SHARD] = dinv[c * SHARD:(c + 1) * SHARD]
        dinvD.append(np.ascontiguousarray(np.tile(dd, (64, 1))))
    meta = dict(C=C, sb_meta=sb_meta, TOTC=TOTC, TOT=TOT)
    return idx_streams, dloc_streams, dinvP, dinvD, meta


def _build_nc(meta):
    import concourse.bacc as bacc
    import concourse.mybir as mybir
    import concourse.tile as tile
    from concourse import library_config

    f32 = mybir.dt.float32
    f16 = mybir.dt.float16
    i16 = mybir.dt.int16
    AF = mybir.ActivationFunctionType
    ALU = mybir.AluOpType
    TOTC, TOT = meta["TOTC"], meta["TOT"]
    C, sb_meta = meta["C"], meta["sb_meta"]
    W3S = TSB3 * 128      # 256, S width

    nc = bacc.Bacc("TRN2")
    xT = nc.dram_tensor("xT", [F, NPAD], f16, kind="ExternalInput")
    wg = nc.dram_tensor("wg", [F, H], f16, kind="ExternalInput")
    w1 = nc.dram_tensor("w1", [64, 32], f32, kind="ExternalInput")
    w2 = nc.dram_tensor("w2", [32, 16], f32, kind="ExternalInput")
    w3 = nc.dram_tensor("w3", [16, 4], f32, kind="ExternalInput")
    bg = nc.dram_tensor("bg", [64, 1], f32, kind="ExternalInput")
    b1 = nc.dram_tensor("b1", [32, 1], f32, kind="ExternalInput")
    b2 = nc.dram_tensor("b2", [16, 1], f32, kind="ExternalInput")
    b3r = nc.dram_tensor("b3r", [1, 4], f32, kind="ExternalInput")
    iotam = nc.dram_tensor("iotam", [128, W3S], f16, kind="ExternalInput")
    onesr = nc.dram_tensor("onesr", [1, 128], f32, kind="ExternalInput")
    idxT = nc.dram_tensor("idx", [128, TOT // 16], i16, kind="ExternalInput")
    dlocT = nc.dram_tensor("dloc", [128, TOTC], f32, kind="ExternalInput")
    dinvPT = nc.dram_tensor("dinvP", [128, NPAD // 128], f32,
                            kind="ExternalInput")
    dinvDT = nc.dram_tensor("dinvD", [64, SPAD], f32, kind="ExternalInput")
    outT = nc.dram_tensor("out", [SPAD, NCLS], f32, kind="ExternalOutput")

    NT1 = NPAD // 128  # 784 phase-1 tiles
    sb1 = [list(range(s, min(s + TSB1, NT1))) for s in range(0, NT1, TSB1)]
    # per-pass (groups 0-1 / groups 2-3) chunk-count maxima for tile sizing
    maxc0 = max(m["goff"][2] for m in sb_meta)
    maxc1 = max(m["totc"] - m["goff"][2] for m in sb_meta)
    maxc = max(maxc0, maxc1)

    with tile.TileContext(nc) as tc:
        with tc.tile_pool(name="const", bufs=1) as cp, \
             tc.tile_pool(name="dram", bufs=1, space="DRAM") as dram:
            # +2/+4 guard rows: the last pair descriptor of a view reads
            # past the table end
            h01 = dram.tile([2 * GSZ + 2, H], f16, tag="h01")
            h23 = dram.tile([2 * GSZ + 2, H], f16, tag="h23")
            hAll = dram.tile([NPAD + 4, H], f16, tag="hAll")
            nc.gpsimd.load_library(library_config.mlp)

            wg0 = cp.tile([128, H], f16, tag="wg0")
            wg1 = cp.tile([128, H], f16, tag="wg1")
            nc.sync.dma_start(wg0[:], wg[0:128, :])
            nc.sync.dma_start(wg1[:], wg[128:256, :])
            w1s = cp.tile([64, 32], f32, tag="w1s")
            w2s = cp.tile([32, 16], f32, tag="w2s")
            w3s = cp.tile([16, 4], f32, tag="w3s")
            bgs = cp.tile([64, 1], f32, tag="bgs")
            b1s = cp.tile([32, 1], f32, tag="b1s")
            b2s = cp.tile([16, 1], f32, tag="b2s")
            b3s = cp.tile([1, 4], f32, tag="b3s")
            iots = cp.tile([128, W3S], f16, tag="iots")
            dps = cp.tile([128, NPAD // 128], f32, tag="dps")
            nc.sync.dma_start(dps[:], dinvPT[:, :])
            ones = cp.tile([1, 128], f32, tag="ones")
            for t_, d_ in ((w1s, w1), (w2s, w2), (w3s, w3), (bgs, bg),
                           (b1s, b1), (b2s, b2), (b3s, b3r), (iots, iotam),
                           (ones, onesr)):
                nc.sync.dma_start(t_[:], d_[:, :])

            with tc.tile_pool(name="p1", bufs=2) as p1p, \
                 tc.tile_pool(name="ps1", bufs=2, space="PSUM") as ps1, \
                 tc.tile_pool(name="p3", bufs=3) as p3p, \
                 tc.tile_pool(name="gb", bufs=3) as gbp, \
                 tc.tile_pool(name="sp", bufs=6) as sp, \
                 tc.tile_pool(name="ep", bufs=3) as ep, \
                 tc.tile_pool(name="oa", bufs=1) as oap, \
                 tc.tile_pool(name="agg", bufs=2, space="PSUM") as aggp, \
                 tc.tile_pool(name="mlp", bufs=3, space="PSUM") as mlpp:
                # -------- phase 1: h = x @ W_gcn, halves written in order ---
                nhalf = len(sb1) // 2
                for bi, tiles in enumerate(sb1):
                    T = len(tiles)
                    t0 = tiles[0]
                    xt0 = p1p.tile([128, TSB1 * 128], f16, tag="xt0")
                    xt1 = p1p.tile([128, TSB1 * 128], f16, tag="xt1")
                    nc.sync.dma_start(
                        xt0[:, :T * 128], xT[0:128, t0 * 128:(t0 + T) * 128])
                    nc.sync.dma_start(
                        xt1[:, :T * 128], xT[128:256, t0 * 128:(t0 + T) * 128])
                    hsb = p1p.tile([128, TSB1, H], f16, tag="hsb")
                    for i in range(T):
                        ps = ps1.tile([128, H], f32, tag="hps")
                        nc.tensor.matmul(ps[:], xt0[:, i * 128:(i + 1) * 128],
                                         wg0[:], start=True, stop=False)
                        nc.tensor.matmul(ps[:], xt1[:, i * 128:(i + 1) * 128],
                                         wg1[:], start=False, stop=True)
                        if i % 2 == 0:
                            nc.scalar.activation(
                                hsb[:, i, :], ps[:], AF.Copy,
                                scale=dps[:, t0 + i:t0 + i + 1])
                        else:
                            nc.vector.tensor_scalar(
                                hsb[:, i, :], ps[:],
                                dps[:, t0 + i:t0 + i + 1], None, op0=ALU.mult)
                    hP = h01 if bi < nhalf else h23
                    r0 = (bi if bi < nhalf else bi - nhalf) * TSB1 * 128
                    # partition-major row order -> contiguous 1 KB runs
                    nc.sync.dma_start(
                        hP[r0:r0 + T * 128, :]
                        .rearrange("(p t) f -> p t f", p=128),
                        hsb[:, :T, :])
                    # second copy: full table for the self-loop cells
                    nc.scalar.dma_start(
                        hAll[bi * TSB1 * 128:bi * TSB1 * 128 + T * 128, :]
                        .rearrange("(p t) f -> p t f", p=128),
                        hsb[:, :T, :])

                # -------- phase 3: two passes (groups 0-1, then 2-3) --------
                outacc = oap.tile([128, NT, NCLS], f32, tag="outacc")
                accT = oap.tile([64, NT * 128], f32, tag="accT")
                for pas in (0, 1):
                    hP = h01 if pas == 0 else h23
                    gl = 2 * pas
                    for m in sb_meta:
                        s = m["sb"]
                        wsb = min(TSB3, NT - s * TSB3) * 128
                        pco = m["goff"][gl]                  # pass col offset
                        pend = m["totc"] if pas else m["goff"][2]
                        ptc = pend - pco                     # pass chunk count
                        cb = m["colbase"] + pco              # global col base
                        idxsb = p3p.tile([128, maxc * 8], i16, tag="idx")
                        nc.sync.dma_start(idxsb[:, :ptc * 8],
                                          idxT[:, cb * 8:(cb + ptc) * 8])
                        dlsb = p3p.tile([128, maxc], f32, tag="dl")
                        nc.sync.dma_start(dlsb[:, :ptc],
                                          dlocT[:, cb:cb + ptc])
                        gbuf = gbp.tile([128, maxc, 128], f16, tag="gbuf")
                        for g in ((gl, gl + 1) if pas == 0
                                  else (2, 3, 4, 5, 6, 7)):
                            L = m["L"][g]
                            if L == 0:
                                continue
                            go = m["goff"][g] - pco
                            if g < 4:
                                # pair view: row k = rows [2k+par, 2k+par+1]
                                par = g & 1
                                hV = hP[:].rearrange("r f -> (r f)")[
                                    par * H:par * H + GSZ * 2 * H].rearrange(
                                    "(r ff) -> r ff", ff=2 * H)
                                step = 2 * H
                            else:
                                # self cells: row k = rows [4k+q, 4k+q+1] of
                                # the full table
                                q = g - 4
                                hV = hAll[:].rearrange("r f -> (r f)")[
                                    q * H:q * H + (NPAD // 4) * 4 * H
                                ].rearrange("(r s ff) -> r s ff",
                                            s=2, ff=2 * H)[:, 0, :]
                                step = 4 * H
                            # SWDGE ring caps one gather at ~1024 idxs
                            for k in range(0, L, 1024):
                                ni = min(1024, L - k)
                                c0 = go + k // 128
                                nc.gpsimd.dma_gather(
                                    gbuf[:, c0:c0 + ni // 128, :],
                                    hV,
                                    idxsb[:, c0 * 8:(c0 + ni // 128) * 8],
                                    ni, ni, 2 * H, elem_step=step)
                        agg = aggp.tile([64, W3S], f32, tag="agg")
                        if ptc == 0:
                            nc.vector.memset(agg[:], 0.0)
                        for j in range(ptc):
                            S = sp.tile([128, W3S], f16, tag="S")
                            nc.vector.tensor_scalar(
                                S[:], iots[:], dlsb[:, j:j + 1], None,
                                op0=ALU.is_equal)
                            nc.tensor.matmul(
                                agg[:], gbuf[:, j, 0:H], S[:],
                                start=(j == 0),
                                stop=(j == ptc - 1))
                        if pas == 0:
                            nc.vector.tensor_copy(
                                accT[:, s * W3S:s * W3S + wsb], agg[:, :wsb])
                            continue
                        ddsb = ep.tile([64, W3S], f32, tag="dd")
                        nc.sync.dma_start(
                            ddsb[:, :wsb], dinvDT[:, s * W3S:s * W3S + wsb])
                        t0p = ep.tile([64, W3S], f32, tag="t0p")
                        nc.vector.tensor_add(
                            t0p[:, :wsb], accT[:, s * W3S:s * W3S + wsb],
                            agg[:, :wsb])
                        t0m = ep.tile([64, W3S], f32, tag="t0m")
                        nc.vector.tensor_mul(t0m[:, :wsb], t0p[:, :wsb],
                                             ddsb[:, :wsb])
                        for ti in range(min(TSB3, NT - s * TSB3)):
                            t = s * TSB3 + ti
                            t0s = ep.tile([64, 128], f32, tag="t0")
                            nc.scalar.activation(
                                t0s[:], t0m[:, ti * 128:(ti + 1) * 128],
                                AF.Relu, bias=bgs[:])
                            pm1 = mlpp.tile([32, 128], f32, tag="pm")
                            nc.tensor.matmul(pm1[:], w1s[:], t0s[:],
                                             start=True, stop=True)
                            t1s = ep.tile([32, 128], f32, tag="t1")
                            nc.scalar.activation(t1s[:], pm1[:], AF.Relu,
                                                 bias=b1s[:])
                            pm2 = mlpp.tile([16, 128], f32, tag="pm")
                            nc.tensor.matmul(pm2[:], w2s[:], t1s[:],
                                             start=True, stop=True)
                            t2s = ep.tile([16, 128], f32, tag="t2")
                            nc.scalar.activation(t2s[:], pm2[:], AF.Relu,
                                                 bias=b2s[:])
                            pm3 = mlpp.tile([128, NCLS], f32, tag="pm")
                            nc.tensor.matmul(pm3[:], t2s[:], w3s[:],
                                             start=True, stop=False)
                            nc.tensor.matmul(pm3[:], ones[:], b3s[:],
                                             start=False, stop=True)
                            nmax = ep.tile([128, 1], f32, tag="nmax")
                            nc.vector.tensor_reduce(nmax[:], pm3[:],
                                                    axis=mybir.AxisListType.X,
                                                    op=ALU.max, negate=True)
                            esb = ep.tile([128, NCLS], f32, tag="esb")
                            ssum = ep.tile([128, 1], f32, tag="ssum")
                            nc.scalar.activation(esb[:], pm3[:], AF.Exp,
                                                 bias=nmax[:],
                                                 accum_out=ssum[:])
                            lsb = ep.tile([128, 1], f32, tag="lsb")
                            nc.scalar.activation(lsb[:], ssum[:], AF.Ln)
                            nc.vector.tensor_scalar(
                                outacc[:, t, :], pm3[:], nmax[:], lsb[:],
                                op0=ALU.add, op1=ALU.subtract)
                nc.sync.dma_start(
                    outT[:, :].rearrange("(t p) c -> p t c", p=128),
                    outacc[:])
    nc.compile()
    return nc


def kernel(x, edge_index, W_gcn, b_gcn, W1, b1, W2, b2, W3, b3,
           _trace=False, _tmpdir=None):
    from concourse.bass_utils import run_bass_kernel_spmd

    x = np.asarray(x, dtype=np.float32)
    idx_streams, dloc_streams, dinvP, dinvD, meta = _host_prep(edge_index)
    nc = _build_nc(meta)

    xTp = np.zeros((F, NPAD), dtype=np.float16)
    xt = np.ascontiguousarray(x.T).astype(np.float16)
    for c in range(NCORES):
        xTp[:, c * SPAD:c * SPAD + SHARD] = xt[:, c * SHARD:(c + 1) * SHARD]
    iot = np.tile(np.arange(TSB3 * 128, dtype=np.float16), (128, 1))
    common = {
        "xT": xTp,
        "wg": np.asarray(W_gcn, np.float16),
        "w1": np.asarray(W1, np.float32),
        "w2": np.asarray(W2, np.float32),
        "w3": np.asarray(W3, np.float32),
        "bg": np.asarray(b_gcn, np.float32).reshape(64, 1),
        "b1": np.asarray(b1, np.float32).reshape(32, 1),
        "b2": np.asarray(b2, np.float32).reshape(16, 1),
        "b3r": np.asarray(b3, np.float32).reshape(1, 4),
        "iotam": iot,
        "onesr": np.ones((1, 128), dtype=np.float32),
        "dinvP": dinvP,
    }
    in_maps = []
    for c in range(NCORES):
        m = dict(common)
        m["idx"] = idx_streams[c]
        m["dloc"] = dloc_streams[c]
        m["dinvD"] = dinvD[c]
        in_maps.append(m)

    res = run_bass_kernel_spmd(nc, in_maps, core_ids=list(range(NCORES)),
                               trace=_trace, tmpdir=_tmpdir)
    out = np.concatenate(
        [res.results[c]["out"][:SHARD] for c in range(NCORES)], axis=0)
    if _trace:
        kernel.last_exec_time_ns = res.exec_time_ns
    return out


kernel.last_exec_time_ns = None
